# revision 15
# baseline (speedup 1.0000x reference)
"""Multi-head attention Trainium2 Bass kernel (8 NeuronCores), v3.

Problem: nn_MultiHeadAttention (B=2, S=2048, D=1024, H=16, DK=64).

The reference's raw `.view(B, H, S, DK)` reshape makes head h of batch b a
reinterpretation of the contiguous 128-row block x[b, 128h:128h+128, :], so
each (b, h) is an independent attention problem. 32 pairs over 8 cores ->
4 pairs/core, no collectives. Query/key positions are permuted
(s2' = m*128 + r instead of r*16 + m) identically on q and k (softmax is
permutation-invariant along keys) and un-permuted for free by the ctx
layout.

PE cycle accounting (measured: PE runs 2.4 GHz, cost = moving-free-dim
cycles, stationary loads hidden when rows <= previous stream): projections
98304c + scores 131072c + ctx 131072c + out-proj 32768c = 393216c = 164 us
— the fp16 floor. The scalar exp wall is ~116 us. v3 aims to keep the PE
FIFO gapless from ~t=3us:

 - one software-pipelined stream: k-proj (kc-streamed behind fine-grained
   DMA pieces, 5 open psums), q c0..3, v(pair0), then 8 attention passes
   (pair x query-half); q c4..7, v(pair1..3) and per-pair output
   projections are emission-interleaved into the passes, filling the PE
   slack under the exp cadence.
 - ctx keeps v1's orientation (lhsT = v65 [128,65]: 65-row weight loads
   hide under 512-cycle streams; the query-major alternative is
   weight-load-bound and no faster).
 - PSUM: one pending accumulation group per bank (hardware constraint,
   verified): sw ring 2x[128,1024] (4 banks) + 2 ctx accumulators
   [65,512] (2 banks, one per query-half pass) + 1 rotating bank for
   interleaved projections/out-proj + 1 spare via the pass split.
 - scalar does exp + phase-1 drains + accumulator drains only; softmax
   reciprocal is ONE [4,512] DVE instruction per pair (v1 burned 53 us
   in 16 of these); gpsimd (SBUF-only engine) does denominator gathers
   and partition broadcasts.
 - fp16 output, cast + bias on host: halves the tail DMA.

fp16 matmul operands (fp32 PSUM); bq/bk/bv are zeros by spec; bo on host.
"""

import sys

sys.path.insert(0, "/opt/trn_rl_repo")

import numpy as np

import concourse.bass as bass  # noqa: E402
import concourse.tile as tile  # noqa: E402
from concourse import bacc, mybir  # noqa: E402
from concourse.bass_utils import run_bass_kernel_spmd  # noqa: E402

F16 = mybir.dt.float16
F32 = mybir.dt.float32

B, S, D, H = 2, 2048, 1024, 16
DK = 64
NCORES = 8
NPAIR = 4
R = 128
NM = 16
S2 = NM * R
KC = D // 128
SCALE = 1.0 / np.sqrt(np.float32(DK))


def _build():
    nc = bacc.Bacc("TRN2", target_bir_lowering=False, debug=False,
                   num_devices=NCORES)

    xTa = nc.dram_tensor("xTa", [128, KC * 512], F16, kind="ExternalInput").ap()
    wqa = nc.dram_tensor("wqa", [128, KC * 1024], F16,
                         kind="ExternalInput").ap()
    wka = nc.dram_tensor("wka", [128, KC * 1024], F16,
                         kind="ExternalInput").ap()
    wva = nc.dram_tensor("wva", [128, KC * 1024], F16,
                         kind="ExternalInput").ap()
    woa = nc.dram_tensor("woa", [128, KC * 1024], F16,
                         kind="ExternalInput").ap()
    out = nc.dram_tensor("out", [NPAIR * R, D], F16, kind="ExternalOutput").ap()

    with tile.TileContext(nc) as tc:
        with tc.tile_pool(name="w", bufs=1) as wpool, \
             tc.tile_pool(name="xp", bufs=1) as xpool, \
             tc.tile_pool(name="qk", bufs=1) as qkpool, \
             tc.tile_pool(name="v6", bufs=1) as vpool, \
             tc.tile_pool(name="pt", bufs=1) as ptpool, \
             tc.tile_pool(name="cu", bufs=1) as cupool, \
             tc.tile_pool(name="cx", bufs=1) as cpool, \
             tc.tile_pool(name="ot", bufs=1) as otpool, \
             tc.tile_pool(name="ps", bufs=1, space="PSUM") as pspool:

            # ---------------- SBUF persistent tiles ----------------
            xT = xpool.tile([128, KC * 512], F16, name="xT", tag="x", bufs=1)
            wk = wpool.tile([128, KC * 1024], F16, name="wk", tag="wk", bufs=1)
            wq = wpool.tile([128, KC * 1024], F16, name="wq", tag="wq", bufs=1)
            wv = wpool.tile([128, KC * 1024], F16, name="wv", tag="wv", bufs=1)
            wo = wpool.tile([128, KC * 1024], F16, name="wo", tag="wo", bufs=1)

            qT2 = [qkpool.tile([128, S2], F16, name=f"qT2{t}", tag=f"q{t}",
                               bufs=1) for t in range(2)]
            kTz = [qkpool.tile([128, S2], F16, name=f"kTz{p}", tag=f"kz{p}",
                               bufs=1) for p in range(NPAIR)]
            v65 = [vpool.tile([128, NM * 65], F16, name=f"v65{p}",
                              tag=f"v{p}", bufs=1) for p in range(NPAIR)]
            ctx = [cpool.tile([128, D], F16, name=f"ctx{p}", tag=f"c{p}",
                              bufs=1) for p in range(NPAIR)]
            ones64 = cupool.tile([97, 64], F16, name="ones64", tag="o64",
                                 bufs=1)
            nc.vector.memset(ones64[:], 1.0)

            # ---------------- input DMA ----------------
            # sync: xT/wk pieces interleaved per kc (k-proj streams behind
            # them), then wo. scalar: wv. gpsimd: wq.
            for kc in range(KC):
                nc.sync.dma_start(xT[:, kc * 512:(kc + 1) * 512],
                                  xTa[:, kc * 512:(kc + 1) * 512])
                nc.sync.dma_start(wk[:, kc * 1024:(kc + 1) * 1024],
                                  wka[:, kc * 1024:(kc + 1) * 1024])
            for i in range(4):
                nc.gpsimd.dma_start(wq[:, i * 2048:(i + 1) * 2048],
                                    wqa[:, i * 2048:(i + 1) * 2048])
            for i in range(4):
                nc.scalar.dma_start(wv[:, i * 2048:(i + 1) * 2048],
                                    wva[:, i * 2048:(i + 1) * 2048])
            for i in range(2):
                nc.sync.dma_start(wo[:, i * 4096:(i + 1) * 4096],
                                  woa[:, i * 4096:(i + 1) * 4096])

            # kTz zero halves (the other pair's q rows multiply zeros) and
            # v65 ones columns (free softmax denominators).
            for p in range(NPAIR):
                other = (1 - (p % 2)) * 64
                eng = nc.vector if p % 2 == 0 else nc.gpsimd
                eng.memset(kTz[p][other:other + 64, :], 0.0)
            for p in range(NPAIR):
                ones_cols = v65[p][:].rearrange("p (m c) -> p m c",
                                                m=NM)[:, :, 64:65]
                nc.gpsimd.memset(ones_cols, 1.0)

            # ---------------- projection helpers ----------------
            def qk_drain(ps, c, dst_tiles, scalar_ok):
                for pr in range(NPAIR):
                    half = (pr % 2) * 64
                    dst_t = dst_tiles[pr // 2] if len(dst_tiles) == 2 \
                        else dst_tiles[pr]
                    for mp in range(2):
                        m = 2 * c + mp
                        src = ps[mp * 64:mp * 64 + 64,
                                 pr * 128:(pr + 1) * 128]
                        dst = dst_t[half:half + 64, m * 128:(m + 1) * 128]
                        if mp == 0 or not scalar_ok:
                            nc.vector.tensor_copy(dst, src)
                        else:
                            nc.scalar.copy(dst, src)

            def proj_qk(w_tile, c, dst_tiles, tag, scalar_ok):
                ps = pspool.tile([128, 512], F32, name=f"pp{c}", tag=tag,
                                 bufs=2 if tag in ("sc", "pj") else 1)
                for kc in range(KC):
                    nc.tensor.matmul(
                        ps[:],
                        w_tile[:, kc * 1024 + c * 128:kc * 1024 + (c + 1) * 128],
                        xT[:, kc * 512:(kc + 1) * 512],
                        start=(kc == 0), stop=(kc == KC - 1))
                qk_drain(ps, c, dst_tiles, scalar_ok)

            def proj_v(pr, g, tag):
                psv = pspool.tile([128, 512], F32, name=f"pv{pr}{g}", tag=tag,
                                  bufs=2 if tag in ("sc", "pj") else 1)
                for kc in range(KC):
                    nc.tensor.matmul(
                        psv[:],
                        xT[:, kc * 512 + pr * 128:kc * 512 + (pr + 1) * 128],
                        wv[:, kc * 1024 + g * 512:kc * 1024 + (g + 1) * 512],
                        start=(kc == 0), stop=(kc == KC - 1))
                dst = v65[pr][:].rearrange("p (m c) -> p m c",
                                           m=NM)[:, g * 8:(g + 1) * 8, 0:64]
                src = psv[:].rearrange("p (m c) -> p m c", m=8)
                nc.vector.tensor_copy(dst, src)

            # ---------------- phase 1 lead-in ----------------
            # k-projection kc-outer for c0..4 (5 open psums) so the PE
            # starts as soon as the first xT/wk DMA pieces land.
            kps = [pspool.tile([128, 512], F32, name=f"kp{c}", tag=tg,
                               bufs=2 if tg in ("sc", "pj") else 1)
                   for c, tg in enumerate(("sc", "sc", "pj", "pcA", "pcB"))]
            for kc in range(KC):
                for c in range(5):
                    nc.tensor.matmul(
                        kps[c][:],
                        wk[:, kc * 1024 + c * 128:kc * 1024 + (c + 1) * 128],
                        xT[:, kc * 512:(kc + 1) * 512],
                        start=(kc == 0), stop=(kc == KC - 1))
            for c in range(5):
                qk_drain(kps[c], c, kTz, scalar_ok=True)
            for c, tg in zip(range(5, 8), ("sc", "sc", "pj")):
                proj_qk(wk, c, kTz, tg, scalar_ok=True)
            for c, tg in zip(range(4), ("pcA", "pcB", "sc", "sc")):
                proj_qk(wq, c, qT2, tg, scalar_ok=True)
            proj_v(0, 0, "pj")
            proj_v(0, 1, "pcA")

            # ---------------- attention: 8 passes (pair x query-half) ----
            cu = {}
            dens = {}
            for pr in range(NPAIR):
                t, half = pr // 2, (pr % 2) * 64
                for h2 in range(2):
                    pc = [pspool.tile([65, 512], F32, name=f"pc{qh}",
                                      tag=("pcA", "pcB")[qh], bufs=1)
                          for qh in range(2)]
                    for mk in range(NM):
                        sw = pspool.tile([128, 1024], F32, name="sw",
                                         tag="sc", bufs=2)
                        for qh in range(2):
                            nc.tensor.matmul(
                                sw[:, qh * 512:(qh + 1) * 512],
                                kTz[pr][:, mk * 128:(mk + 1) * 128],
                                qT2[t][:, (h2 * 1024 + qh * 512):
                                       (h2 * 1024 + (qh + 1) * 512)],
                                start=True, stop=True)
                        pT = ptpool.tile([128, 1024], F16, name="pT",
                                         tag="pt", bufs=3)
                        nc.scalar.activation(
                            pT[:], sw[:], mybir.ActivationFunctionType.Exp,
                            scale=float(SCALE))
                        for qh in range(2):
                            nc.tensor.matmul(
                                pc[qh][:],
                                v65[pr][:, mk * 65:(mk + 1) * 65],
                                pT[:, qh * 512:(qh + 1) * 512],
                                start=(mk == 0), stop=(mk == NM - 1))
                        # ---- interleaved projections (fill PE slack) ----
                        if pr == 0 and h2 == 0 and mk in (2, 5, 8, 11):
                            proj_qk(wq, 4 + (2, 5, 8, 11).index(mk), qT2,
                                    "pj", scalar_ok=False)
                        if h2 == 1 and pr < NPAIR - 1 and mk in (2, 8):
                            proj_v(pr + 1, 0 if mk == 2 else 1, "pj")
                    # ---- pass tail: drain accumulators to SBUF ----
                    # (den rows staged at 32-aligned partitions of one tile
                    # so a single batched reciprocal covers the pair)
                    if h2 == 0:
                        den = cupool.tile([97, 512], F32, name="den",
                                          tag="den", bufs=2)
                        dens[pr] = den
                    for qh in range(2):
                        qs = h2 * 2 + qh
                        c_t = cupool.tile([64, 512], F32,
                                          name=f"cu{pr}{h2}{qh}",
                                          tag=f"cu{qs}", bufs=2)
                        nc.scalar.copy(c_t[:], pc[qh][0:64, :])
                        nc.scalar.copy(dens[pr][qs * 32:qs * 32 + 1, :],
                                       pc[qh][64:65, :])
                        cu[(pr, qs)] = c_t

                # ---- pair tail: normalize into ctx, then out-proj ----
                rec = cupool.tile([97, 512], F32, name="rec", tag="rec",
                                  bufs=2)
                nc.vector.reciprocal(rec[:], dens[pr][:])
                rec16 = cupool.tile([97, 512], F16, name="rec16", tag="r16",
                                    bufs=2)
                nc.vector.tensor_copy(rec16[:], rec[:])
                # PE operand base partition must be 0/32/64: relay qs=3's
                # reciprocal row (partition 96) to an offset-0 tile.
                rec16b = cupool.tile([1, 512], F16, name="rec16b", tag="r16b",
                                     bufs=2)
                nc.vector.tensor_copy(rec16b[:], rec16[96:97, :])
                for qs in range(4):
                    pbs = pspool.tile([64, 512], F32, name="pbs", tag="pj",
                                      bufs=2)
                    lo = qs * 32 if qs < 3 else 0
                    rsrc = rec16[lo:lo + 1, :] if qs < 3 else rec16b[:]
                    nc.tensor.matmul(pbs[:], ones64[lo:lo + 1, :], rsrc,
                                     start=True, stop=True)
                    for p2 in range(2):
                        src = cu[(pr, qs)][:].rearrange(
                            "p (a q c) -> p a q c", a=2, q=2)[:, :, p2, :]
                        bb = pbs[:].rearrange(
                            "p (a q c) -> p a q c", a=2, q=2)[:, :, p2, :]
                        dst = ctx[pr][p2 * 64:(p2 + 1) * 64,
                                      qs * 256:(qs + 1) * 256].rearrange(
                            "p (a c) -> p a c", a=2)
                        nc.vector.tensor_mul(dst, src, bb)
                for jb in range(2):
                    po = pspool.tile([128, 512], F32, name="po", tag="pj",
                                     bufs=2)
                    for c in range(KC):
                        nc.tensor.matmul(
                            po[:],
                            ctx[pr][:, c * 128:(c + 1) * 128],
                            wo[:, c * 1024 + jb * 512:
                               c * 1024 + (jb + 1) * 512],
                            start=(c == 0), stop=(c == KC - 1))
                    ot = otpool.tile([128, 512], F16, name="ot", tag="ot",
                                     bufs=4)
                    nc.vector.tensor_copy(ot[:], po[:])
                    nc.sync.dma_start(
                        out[pr * 128:(pr + 1) * 128,
                            jb * 512:(jb + 1) * 512], ot[:])

    nc.compile()
    return nc


_CACHE = {}


def _get_nc():
    if "nc" not in _CACHE:
        _CACHE["nc"] = _build()
    return _CACHE["nc"]


def _kc_block(a, cols):
    """[1024, cols] -> [128, 8*cols] with kc blocks along columns."""
    return np.ascontiguousarray(
        a.reshape(KC, 128, cols).transpose(1, 0, 2).reshape(128, KC * cols))


def _prep_inputs(x, Wq, Wk, Wv, Wo):
    x = np.asarray(x, dtype=np.float32)
    wqa = _kc_block(np.ascontiguousarray(Wq.T, dtype=np.float16), 1024)
    wka = _kc_block(np.ascontiguousarray(Wk.T, dtype=np.float16), 1024)
    wva = _kc_block(np.ascontiguousarray(Wv.T, dtype=np.float16), 1024)
    woa = _kc_block(np.ascontiguousarray(Wo.T, dtype=np.float16), 1024)

    in_maps = []
    for core in range(NCORES):
        b, hg = core // 4, core % 4
        rows = x[b, hg * 512:(hg + 1) * 512, :]
        xTa = _kc_block(np.ascontiguousarray(rows.T.astype(np.float16)), 512)
        in_maps.append({
            "xTa": xTa, "wqa": wqa, "wka": wka, "wva": wva, "woa": woa,
        })
    return in_maps


def _run(in_maps, trace=False):
    nc = _get_nc()
    return run_bass_kernel_spmd(nc, in_maps, core_ids=list(range(NCORES)),
                                trace=trace)


def kernel(x, Wq, bq, Wk, bk, Wv, bv, Wo, bo, _trace=False):
    x = np.asarray(x, dtype=np.float32)
    in_maps = _prep_inputs(x, np.asarray(Wq), np.asarray(Wk),
                           np.asarray(Wv), np.asarray(Wo))
    res = _run(in_maps, trace=_trace)
    out = np.empty((B, S, D), dtype=np.float32)
    for core in range(NCORES):
        b, hg = core // 4, core % 4
        out[b, hg * 512:(hg + 1) * 512, :] = res.results[core]["out"]
    out += np.asarray(bo, dtype=np.float32)[None, None, :]
    kernel.last_result = res
    return out


# revision 16
# speedup vs baseline: 1.0646x; 1.0646x over previous
"""Multi-head attention Trainium2 Bass kernel (8 NeuronCores), v3.1.

Problem: nn_MultiHeadAttention (B=2, S=2048, D=1024, H=16, DK=64).

The reference's raw `.view(B, H, S, DK)` reshape makes head h of batch b a
reinterpretation of the contiguous 128-row block x[b, 128h:128h+128, :], so
each (b, h) is an independent attention problem. 32 pairs over 8 cores ->
4 pairs/core, no collectives. Query/key positions are permuted
(s2' = m*128 + r instead of r*16 + m) identically on q and k (softmax is
permutation-invariant along keys) and un-permuted for free by the ctx
layout.

PE cycle floor (fp16, 2.4 GHz, cost = moving-free-dim cycles): projections
98304c + scores 131072c + ctx 131072c + out-proj 32768c ~= 164 us. Scalar
exp wall ~116 us. The kernel is one software-pipelined stream built to
keep the PE FIFO dense:

 - DMA in feature-major pieces on two queues (sync: x, Wk lo, Wv lo,
   Wk hi, Wv hi, Wo; gpsimd: Wq lo/hi), so attention on pair 0 starts
   after ~4 MB instead of the full 9.4 MB.
 - k/q c0..3, v(pair0) lo-half first, then 8 attention passes
   (pair x query-half, 16 key-chunks each). Remaining projection chunks,
   the deferred normalize/broadcast, and per-pair output projections are
   emission-interleaved at mk hooks inside later passes, so no PE
   instruction ever waits on the DVE normalize chain.
 - ctx accumulates v1-style (lhsT = v65 [128,65], 65-row weight loads
   hide under 512-cycle streams; the query-major alternative is
   weight-load-bound). PSUM: one pending accumulation group per bank
   (verified hw constraint): sw ring 2x[128,1024] + pcA/pcB accumulator
   banks + pj ring (2 banks) for projections/out-proj/broadcasts.
 - softmax: v65 ones column makes pc row 64 the denominator; den rows
   drain to 32-aligned partitions of one tile; ONE exact [97,512] DVE
   reciprocal per pair; broadcast down 64 partitions via PE outer
   product (ones[1,64].T @ rec16[1,512] -> PSUM, 213 ns); DVE multiplies
   into the ctx layout the out-projection consumes directly.
 - scalar engine: exp (+ phase-1/den drains only). fp16 output, cast +
   bias on host.

fp16 matmul operands (fp32 PSUM); bq/bk/bv are zeros by spec; bo on host.
"""

import sys

sys.path.insert(0, "/opt/trn_rl_repo")

import numpy as np

import concourse.bass as bass  # noqa: E402
import concourse.tile as tile  # noqa: E402
from concourse import bacc, mybir  # noqa: E402
from concourse.bass_utils import run_bass_kernel_spmd  # noqa: E402

F16 = mybir.dt.float16
F32 = mybir.dt.float32

B, S, D, H = 2, 2048, 1024, 16
DK = 64
NCORES = 8
NPAIR = 4
R = 128
NM = 16
S2 = NM * R
KC = D // 128
SCALE = 1.0 / np.sqrt(np.float32(DK))


def _build():
    nc = bacc.Bacc("TRN2", target_bir_lowering=False, debug=False,
                   num_devices=NCORES)

    xTa = nc.dram_tensor("xTa", [128, KC * 512], F16, kind="ExternalInput").ap()
    wqa = nc.dram_tensor("wqa", [128, KC * 1024], F16,
                         kind="ExternalInput").ap()
    wka = nc.dram_tensor("wka", [128, KC * 1024], F16,
                         kind="ExternalInput").ap()
    wva = nc.dram_tensor("wva", [128, KC * 1024], F16,
                         kind="ExternalInput").ap()
    woa = nc.dram_tensor("woa", [128, KC * 1024], F16,
                         kind="ExternalInput").ap()
    out = nc.dram_tensor("out", [NPAIR * R, D], F16, kind="ExternalOutput").ap()

    with tile.TileContext(nc) as tc:
        with tc.tile_pool(name="w", bufs=1) as wpool, \
             tc.tile_pool(name="xp", bufs=1) as xpool, \
             tc.tile_pool(name="qk", bufs=1) as qkpool, \
             tc.tile_pool(name="v6", bufs=1) as vpool, \
             tc.tile_pool(name="pt", bufs=1) as ptpool, \
             tc.tile_pool(name="cu", bufs=1) as cupool, \
             tc.tile_pool(name="cx", bufs=1) as cpool, \
             tc.tile_pool(name="ot", bufs=1) as otpool, \
             tc.tile_pool(name="ps", bufs=1, space="PSUM") as pspool:

            # ---------------- SBUF persistent tiles ----------------
            xT = xpool.tile([128, KC * 512], F16, name="xT", tag="x", bufs=1)
            wk = wpool.tile([128, KC * 1024], F16, name="wk", tag="wk", bufs=1)
            wq = wpool.tile([128, KC * 1024], F16, name="wq", tag="wq", bufs=1)
            wv = wpool.tile([128, KC * 1024], F16, name="wv", tag="wv", bufs=1)
            wo = wpool.tile([128, KC * 1024], F16, name="wo", tag="wo", bufs=1)

            qT2 = [qkpool.tile([128, S2], F16, name=f"qT2{t}", tag=f"q{t}",
                               bufs=1) for t in range(2)]
            kTz = [qkpool.tile([128, S2], F16, name=f"kTz{p}", tag=f"kz{p}",
                               bufs=1) for p in range(NPAIR)]
            v65 = [vpool.tile([128, NM * 65], F16, name=f"v65{p}",
                              tag=f"v{p}", bufs=1) for p in range(NPAIR)]
            ctx = [cpool.tile([128, D], F16, name=f"ctx{p}", tag=f"c{p}",
                              bufs=1) for p in range(NPAIR)]
            ones64 = cupool.tile([97, 64], F16, name="ones64", tag="o64",
                                 bufs=1)
            nc.vector.memset(ones64[:], 1.0)

            # ---------------- input DMA ----------------
            # feature-major halves: lo = output features 0..511 of each kc
            # block (enough for c0..3 / g0 / key-chunks 0..7), hi = rest.
            def fhalf(w_ap, lo):
                return w_ap.rearrange("p (kc f) -> p kc f", kc=KC)[
                    :, :, lo * 512:(lo + 1) * 512]

            for i in range(2):
                nc.sync.dma_start(xT[:, i * 2048:(i + 1) * 2048],
                                  xTa[:, i * 2048:(i + 1) * 2048])
            nc.sync.dma_start(fhalf(wk[:], 0), fhalf(wka, 0))
            nc.gpsimd.dma_start(fhalf(wq[:], 0), fhalf(wqa, 0))
            nc.sync.dma_start(fhalf(wv[:], 0), fhalf(wva, 0))
            nc.sync.dma_start(fhalf(wk[:], 1), fhalf(wka, 1))
            nc.sync.dma_start(fhalf(wv[:], 1), fhalf(wva, 1))
            nc.gpsimd.dma_start(fhalf(wq[:], 1), fhalf(wqa, 1))
            nc.sync.dma_start(wo[:], woa)

            # kTz zero halves (the other pair's q rows multiply zeros) and
            # v65 ones columns (free softmax denominators).
            for p in range(NPAIR):
                other = (1 - (p % 2)) * 64
                eng = nc.vector if p % 2 == 0 else nc.gpsimd
                eng.memset(kTz[p][other:other + 64, :], 0.0)
            for p in range(NPAIR):
                ones_cols = v65[p][:].rearrange("p (m c) -> p m c",
                                                m=NM)[:, :, 64:65]
                nc.gpsimd.memset(ones_cols, 1.0)

            # ---------------- projection helpers ----------------
            def proj_qk(w_tile, c, dst_tiles, tag, scalar_ok):
                ps = pspool.tile([128, 512], F32, name=f"pp{c}", tag=tag,
                                 bufs=2 if tag in ("sc", "pj") else 1)
                for kc in range(KC):
                    nc.tensor.matmul(
                        ps[:],
                        w_tile[:, kc * 1024 + c * 128:kc * 1024 + (c + 1) * 128],
                        xT[:, kc * 512:(kc + 1) * 512],
                        start=(kc == 0), stop=(kc == KC - 1))
                for pr in range(NPAIR):
                    half_p = (pr % 2) * 64
                    dst_t = dst_tiles[pr // 2] if len(dst_tiles) == 2 \
                        else dst_tiles[pr]
                    for mp in range(2):
                        m = 2 * c + mp
                        src = ps[mp * 64:mp * 64 + 64,
                                 pr * 128:(pr + 1) * 128]
                        dst = dst_t[half_p:half_p + 64,
                                    m * 128:(m + 1) * 128]
                        if mp == 0 or not scalar_ok:
                            nc.vector.tensor_copy(dst, src)
                        else:
                            nc.scalar.copy(dst, src)

            def proj_v(pr, g, tag):
                psv = pspool.tile([128, 512], F32, name=f"pv{pr}{g}", tag=tag,
                                  bufs=2 if tag in ("sc", "pj") else 1)
                for kc in range(KC):
                    nc.tensor.matmul(
                        psv[:],
                        xT[:, kc * 512 + pr * 128:kc * 512 + (pr + 1) * 128],
                        wv[:, kc * 1024 + g * 512:kc * 1024 + (g + 1) * 512],
                        start=(kc == 0), stop=(kc == KC - 1))
                dst = v65[pr][:].rearrange("p (m c) -> p m c",
                                           m=NM)[:, g * 8:(g + 1) * 8, 0:64]
                src = psv[:].rearrange("p (m c) -> p m c", m=8)
                nc.vector.tensor_copy(dst, src)

            # ---------------- deferred pair-tail work ----------------
            cu = {}
            dens = {}
            state = {}

            def norm_recip(pr):
                """Batched reciprocal of pair pr's denominators + fp16
                cast + qs=3 relay (PE operand base must be 0/32/64)."""
                rec = cupool.tile([97, 512], F32, name="rec", tag="rec",
                                  bufs=2)
                nc.vector.reciprocal(rec[:], dens[pr][:])
                rec16 = cupool.tile([97, 512], F16, name="rec16", tag="r16",
                                    bufs=2)
                nc.vector.tensor_copy(rec16[:], rec[:])
                rec16b = cupool.tile([1, 512], F16, name="rec16b",
                                     tag="r16b", bufs=2)
                nc.vector.tensor_copy(rec16b[:], rec16[96:97, :])
                state[pr] = (rec16, rec16b)

            def norm_qs(pr, qs):
                """PE outer-product broadcast of 1/den + DVE multiply into
                the ctx layout."""
                rec16, rec16b = state[pr]
                pbs = pspool.tile([64, 512], F32, name="pbs", tag="pj",
                                  bufs=2)
                lo = qs * 32 if qs < 3 else 0
                rsrc = rec16[lo:lo + 1, :] if qs < 3 else rec16b[:]
                nc.tensor.matmul(pbs[:], ones64[lo:lo + 1, :], rsrc,
                                 start=True, stop=True)
                for p2 in range(2):
                    src = cu[(pr, qs)][:].rearrange(
                        "p (a q c) -> p a q c", a=2, q=2)[:, :, p2, :]
                    bb = pbs[:].rearrange(
                        "p (a q c) -> p a q c", a=2, q=2)[:, :, p2, :]
                    dst = ctx[pr][p2 * 64:(p2 + 1) * 64,
                                  qs * 256:(qs + 1) * 256].rearrange(
                        "p (a c) -> p a c", a=2)
                    nc.vector.tensor_mul(dst, src, bb)

            def outproj(pr, jb):
                po = pspool.tile([128, 512], F32, name="po", tag="pj",
                                 bufs=2)
                for c in range(KC):
                    nc.tensor.matmul(
                        po[:],
                        ctx[pr][:, c * 128:(c + 1) * 128],
                        wo[:, c * 1024 + jb * 512:c * 1024 + (jb + 1) * 512],
                        start=(c == 0), stop=(c == KC - 1))
                ot = otpool.tile([128, 512], F16, name="ot", tag="ot",
                                 bufs=4)
                nc.vector.tensor_copy(ot[:], po[:])
                nc.sync.dma_start(
                    out[pr * 128:(pr + 1) * 128,
                        jb * 512:(jb + 1) * 512], ot[:])

            # ---------------- phase 1 lead-in ----------------
            for c, tg in zip(range(4), ("sc", "sc", "pj", "pj")):
                proj_qk(wk, c, kTz, tg, scalar_ok=True)
            for c, tg in zip(range(4), ("pcA", "pcB", "sc", "sc")):
                proj_qk(wq, c, qT2, tg, scalar_ok=True)
            proj_v(0, 0, "pj")

            # ---------------- attention: 8 passes + interleave hooks ----
            hooks = {(p, h): {} for p in range(NPAIR) for h in range(2)}

            def add_hook(pr, h2, mk, fn):
                hooks[(pr, h2)].setdefault(mk, []).append(fn)

            # remaining projections, spread under the exp wall; hi-half
            # key chunks (k c4..7) are consumed from mk=8 of every pass.
            for i, c in enumerate(range(4, 8)):
                add_hook(0, 0, 1 + 2 * i,
                         lambda c=c: proj_qk(wk, c, kTz, "pj",
                                             scalar_ok=False))
            for i, c in enumerate(range(4, 8)):
                add_hook(0, 0, 8 + 2 * i,
                         lambda c=c: proj_qk(wq, c, qT2, "pj",
                                             scalar_ok=False))
            add_hook(0, 0, 3, lambda: proj_v(0, 1, "pj"))
            for p in range(1, NPAIR):
                add_hook(p - 1, 0, 13, lambda p=p: proj_v(p, 0, "pj"))
                add_hook(p - 1, 1, 5, lambda p=p: proj_v(p, 1, "pj"))

            # previous pair's normalize + output projection, slotted in
            # after its DVE reciprocal chain has had time to finish
            for pr in range(NPAIR - 1):
                into = (pr + 1, 0)
                add_hook(*into, 3, lambda pr=pr: (norm_qs(pr, 0),
                                                  norm_qs(pr, 1)))
                add_hook(*into, 5, lambda pr=pr: (norm_qs(pr, 2),
                                                  norm_qs(pr, 3)))
                add_hook(*into, 7, lambda pr=pr: outproj(pr, 0))
                add_hook(*into, 10, lambda pr=pr: outproj(pr, 1))

            for pr in range(NPAIR):
                t = pr // 2
                for h2 in range(2):
                    pc = [pspool.tile([65, 512], F32, name=f"pc{qh}",
                                      tag=("pcA", "pcB")[qh], bufs=1)
                          for qh in range(2)]
                    for mk in range(NM):
                        sw = pspool.tile([128, 1024], F32, name="sw",
                                         tag="sc", bufs=2)
                        for qh in range(2):
                            nc.tensor.matmul(
                                sw[:, qh * 512:(qh + 1) * 512],
                                kTz[pr][:, mk * 128:(mk + 1) * 128],
                                qT2[t][:, (h2 * 1024 + qh * 512):
                                       (h2 * 1024 + (qh + 1) * 512)],
                                start=True, stop=True)
                        pT = ptpool.tile([128, 1024], F16, name="pT",
                                         tag="pt", bufs=3)
                        nc.scalar.activation(
                            pT[:], sw[:], mybir.ActivationFunctionType.Exp,
                            scale=float(SCALE))
                        for qh in range(2):
                            nc.tensor.matmul(
                                pc[qh][:],
                                v65[pr][:, mk * 65:(mk + 1) * 65],
                                pT[:, qh * 512:(qh + 1) * 512],
                                start=(mk == 0), stop=(mk == NM - 1))
                        for fn in hooks[(pr, h2)].get(mk, []):
                            fn()
                    # ---- pass tail: drain accumulators ----
                    # (den rows land at 32-aligned partitions of one tile
                    # so a single batched reciprocal covers the pair)
                    if h2 == 0:
                        dens[pr] = cupool.tile([97, 512], F32, name="den",
                                               tag="den", bufs=2)
                    for qh in range(2):
                        qs = h2 * 2 + qh
                        c_t = cupool.tile([64, 512], F32,
                                          name=f"cu{pr}{h2}{qh}",
                                          tag=f"cu{qs}", bufs=2)
                        nc.vector.tensor_copy(c_t[:], pc[qh][0:64, :])
                        nc.scalar.copy(dens[pr][qs * 32:qs * 32 + 1, :],
                                       pc[qh][64:65, :])
                        cu[(pr, qs)] = c_t
                # pair done: kick off the reciprocal chain (DVE); its
                # consumers are hooked into the next pair's stream.
                norm_recip(pr)

            # last pair's tail runs at the end
            for qs in range(4):
                norm_qs(3, qs)
            outproj(3, 0)
            outproj(3, 1)

    nc.compile()
    return nc


_CACHE = {}


def _get_nc():
    if "nc" not in _CACHE:
        _CACHE["nc"] = _build()
    return _CACHE["nc"]


def _kc_block(a, cols):
    """[1024, cols] -> [128, 8*cols] with kc blocks along columns."""
    return np.ascontiguousarray(
        a.reshape(KC, 128, cols).transpose(1, 0, 2).reshape(128, KC * cols))


def _prep_inputs(x, Wq, Wk, Wv, Wo):
    x = np.asarray(x, dtype=np.float32)
    wqa = _kc_block(np.ascontiguousarray(Wq.T, dtype=np.float16), 1024)
    wka = _kc_block(np.ascontiguousarray(Wk.T, dtype=np.float16), 1024)
    wva = _kc_block(np.ascontiguousarray(Wv.T, dtype=np.float16), 1024)
    woa = _kc_block(np.ascontiguousarray(Wo.T, dtype=np.float16), 1024)

    in_maps = []
    for core in range(NCORES):
        b, hg = core // 4, core % 4
        rows = x[b, hg * 512:(hg + 1) * 512, :]
        xTa = _kc_block(np.ascontiguousarray(rows.T.astype(np.float16)), 512)
        in_maps.append({
            "xTa": xTa, "wqa": wqa, "wka": wka, "wva": wva, "woa": woa,
        })
    return in_maps


def _run(in_maps, trace=False):
    nc = _get_nc()
    return run_bass_kernel_spmd(nc, in_maps, core_ids=list(range(NCORES)),
                                trace=trace)


def kernel(x, Wq, bq, Wk, bk, Wv, bv, Wo, bo, _trace=False):
    x = np.asarray(x, dtype=np.float32)
    in_maps = _prep_inputs(x, np.asarray(Wq), np.asarray(Wk),
                           np.asarray(Wv), np.asarray(Wo))
    res = _run(in_maps, trace=_trace)
    out = np.empty((B, S, D), dtype=np.float32)
    for core in range(NCORES):
        b, hg = core // 4, core % 4
        out[b, hg * 512:(hg + 1) * 512, :] = res.results[core]["out"]
    out += np.asarray(bo, dtype=np.float32)[None, None, :]
    kernel.last_result = res
    return out


# revision 18
# speedup vs baseline: 1.1030x; 1.0361x over previous
"""Multi-head attention Trainium2 Bass kernel (8 NeuronCores), v3.1.

Problem: nn_MultiHeadAttention (B=2, S=2048, D=1024, H=16, DK=64).

The reference's raw `.view(B, H, S, DK)` reshape makes head h of batch b a
reinterpretation of the contiguous 128-row block x[b, 128h:128h+128, :], so
each (b, h) is an independent attention problem. 32 pairs over 8 cores ->
4 pairs/core, no collectives. Query/key positions are permuted
(s2' = m*128 + r instead of r*16 + m) identically on q and k (softmax is
permutation-invariant along keys) and un-permuted for free by the ctx
layout.

PE cycle floor (fp16, 2.4 GHz, cost = moving-free-dim cycles): projections
98304c + scores 131072c + ctx 131072c + out-proj 32768c ~= 164 us. Scalar
exp wall ~116 us. The kernel is one software-pipelined stream built to
keep the PE FIFO dense:

 - DMA in feature-major pieces on two queues (sync: x, Wk lo, Wv lo,
   Wk hi, Wv hi, Wo; gpsimd: Wq lo/hi), so attention on pair 0 starts
   after ~4 MB instead of the full 9.4 MB.
 - k/q c0..3, v(pair0) lo-half first, then 8 attention passes
   (pair x query-half, 16 key-chunks each). Remaining projection chunks,
   the deferred normalize/broadcast, and per-pair output projections are
   emission-interleaved at mk hooks inside later passes, so no PE
   instruction ever waits on the DVE normalize chain.
 - ctx accumulates v1-style (lhsT = v65 [128,65], 65-row weight loads
   hide under 512-cycle streams; the query-major alternative is
   weight-load-bound). PSUM: one pending accumulation group per bank
   (verified hw constraint): sw ring 2x[128,1024] + pcA/pcB accumulator
   banks + pj ring (2 banks) for projections/out-proj/broadcasts.
 - softmax: v65 ones column makes pc row 64 the denominator; den rows
   drain to 32-aligned partitions of one tile; ONE exact [97,512] DVE
   reciprocal per pair; broadcast down 64 partitions via PE outer
   product (ones[1,64].T @ rec16[1,512] -> PSUM, 213 ns); DVE multiplies
   into the ctx layout the out-projection consumes directly.
 - scalar engine: exp (+ phase-1/den drains only). fp16 output, cast +
   bias on host.

fp16 matmul operands (fp32 PSUM); bq/bk/bv are zeros by spec; bo on host.
"""

import sys

sys.path.insert(0, "/opt/trn_rl_repo")

import numpy as np

import concourse.bass as bass  # noqa: E402
import concourse.tile as tile  # noqa: E402
from concourse import bacc, mybir  # noqa: E402
from concourse.bass_utils import run_bass_kernel_spmd  # noqa: E402

F16 = mybir.dt.float16
F32 = mybir.dt.float32

B, S, D, H = 2, 2048, 1024, 16
DK = 64
NCORES = 8
NPAIR = 4
R = 128
NM = 16
S2 = NM * R
KC = D // 128
SCALE = 1.0 / np.sqrt(np.float32(DK))


def _build():
    nc = bacc.Bacc("TRN2", target_bir_lowering=False, debug=False,
                   num_devices=NCORES)

    xTa = nc.dram_tensor("xTa", [128, KC * 512], F16, kind="ExternalInput").ap()
    wqa = nc.dram_tensor("wqa", [128, KC * 1024], F16,
                         kind="ExternalInput").ap()
    wka = nc.dram_tensor("wka", [128, KC * 1024], F16,
                         kind="ExternalInput").ap()
    wva = nc.dram_tensor("wva", [128, KC * 1024], F16,
                         kind="ExternalInput").ap()
    woa = nc.dram_tensor("woa", [128, KC * 1024], F16,
                         kind="ExternalInput").ap()
    out = nc.dram_tensor("out", [NPAIR * R, D], F16, kind="ExternalOutput").ap()

    with tile.TileContext(nc) as tc:
        with tc.tile_pool(name="w", bufs=1) as wpool, \
             tc.tile_pool(name="xp", bufs=1) as xpool, \
             tc.tile_pool(name="qk", bufs=1) as qkpool, \
             tc.tile_pool(name="v6", bufs=1) as vpool, \
             tc.tile_pool(name="pt", bufs=1) as ptpool, \
             tc.tile_pool(name="cu", bufs=1) as cupool, \
             tc.tile_pool(name="cx", bufs=1) as cpool, \
             tc.tile_pool(name="ot", bufs=1) as otpool, \
             tc.tile_pool(name="ps", bufs=1, space="PSUM") as pspool:

            # ---------------- SBUF persistent tiles ----------------
            xT = xpool.tile([128, KC * 512], F16, name="xT", tag="x", bufs=1)
            wk = wpool.tile([128, KC * 1024], F16, name="wk", tag="wk", bufs=1)
            wq = wpool.tile([128, KC * 1024], F16, name="wq", tag="wq", bufs=1)
            wv = wpool.tile([128, KC * 1024], F16, name="wv", tag="wv", bufs=1)
            wo = wpool.tile([128, KC * 1024], F16, name="wo", tag="wo", bufs=1)

            qT2 = [qkpool.tile([128, S2], F16, name=f"qT2{t}", tag=f"q{t}",
                               bufs=1) for t in range(2)]
            kTz = [qkpool.tile([128, S2], F16, name=f"kTz{p}", tag=f"kz{p}",
                               bufs=1) for p in range(NPAIR)]
            v65 = [vpool.tile([128, NM * 65], F16, name=f"v65{p}",
                              tag=f"v{p}", bufs=1) for p in range(NPAIR)]
            ctx = [cpool.tile([128, D], F16, name=f"ctx{p}", tag=f"c{p}",
                              bufs=1) for p in range(NPAIR)]
            ones64 = cupool.tile([97, 64], F16, name="ones64", tag="o64",
                                 bufs=1)
            nc.vector.memset(ones64[:], 1.0)

            # ---------------- input DMA ----------------
            # feature-major halves: lo = output features 0..511 of each kc
            # block (enough for c0..3 / g0 / key-chunks 0..7), hi = rest.
            def fhalf(w_ap, lo):
                return w_ap.rearrange("p (kc f) -> p kc f", kc=KC)[
                    :, :, lo * 512:(lo + 1) * 512]

            for i in range(2):
                nc.sync.dma_start(xT[:, i * 2048:(i + 1) * 2048],
                                  xTa[:, i * 2048:(i + 1) * 2048])
            def fq(w_ap, lo, kc0, kc1):
                return w_ap.rearrange("p (kc f) -> p kc f", kc=KC)[
                    :, kc0:kc1, lo * 512:(lo + 1) * 512]

            nc.sync.dma_start(fq(wk[:], 0, 0, 4), fq(wka, 0, 0, 4))
            nc.sync.dma_start(fq(wk[:], 0, 4, 8), fq(wka, 0, 4, 8))
            nc.gpsimd.dma_start(fhalf(wq[:], 0), fhalf(wqa, 0))
            nc.sync.dma_start(fhalf(wv[:], 0), fhalf(wva, 0))
            nc.sync.dma_start(fhalf(wk[:], 1), fhalf(wka, 1))
            nc.sync.dma_start(fhalf(wv[:], 1), fhalf(wva, 1))
            nc.gpsimd.dma_start(fhalf(wq[:], 1), fhalf(wqa, 1))
            nc.sync.dma_start(wo[:], woa)

            # kTz zero halves (the other pair's q rows multiply zeros) and
            # v65 ones columns (free softmax denominators).
            for p in range(NPAIR):
                other = (1 - (p % 2)) * 64
                eng = nc.vector if p % 2 == 0 else nc.gpsimd
                eng.memset(kTz[p][other:other + 64, :], 0.0)
            for p in range(NPAIR):
                ones_cols = v65[p][:].rearrange("p (m c) -> p m c",
                                                m=NM)[:, :, 64:65]
                nc.gpsimd.memset(ones_cols, 1.0)

            # ---------------- projection helpers ----------------
            chip_ps = {}

            def qk_chip(w_tile, c, dst_tiles, tag, kc0, kc1, scalar_ok=False):
                """Emit kc0..kc1 of one qk projection chunk; drains on the
                last chip."""
                key = (id(w_tile), c)
                if kc0 == 0:
                    chip_ps[key] = pspool.tile(
                        [128, 512], F32, name=f"pp{c}", tag=tag,
                        bufs=2 if tag in ("sc", "pj") else 1)
                ps = chip_ps[key]
                for kc in range(kc0, kc1):
                    nc.tensor.matmul(
                        ps[:],
                        w_tile[:, kc * 1024 + c * 128:kc * 1024 + (c + 1) * 128],
                        xT[:, kc * 512:(kc + 1) * 512],
                        start=(kc == 0), stop=(kc == KC - 1))
                if kc1 == KC:
                    qk_drain(ps, c, dst_tiles, scalar_ok)

            def qk_drain(ps, c, dst_tiles, scalar_ok):
                for pr in range(NPAIR):
                    half_p = (pr % 2) * 64
                    dst_t = dst_tiles[pr // 2] if len(dst_tiles) == 2 \
                        else dst_tiles[pr]
                    for mp in range(2):
                        m = 2 * c + mp
                        src = ps[mp * 64:mp * 64 + 64,
                                 pr * 128:(pr + 1) * 128]
                        dst = dst_t[half_p:half_p + 64,
                                    m * 128:(m + 1) * 128]
                        if mp == 0 or not scalar_ok:
                            nc.vector.tensor_copy(dst, src)
                        else:
                            nc.scalar.copy(dst, src)

            def v_chip(pr, g, tag, kc0, kc1):
                key = ("v", pr, g)
                if kc0 == 0:
                    chip_ps[key] = pspool.tile(
                        [128, 512], F32, name=f"pv{pr}{g}", tag=tag,
                        bufs=2 if tag in ("sc", "pj") else 1)
                psv = chip_ps[key]
                for kc in range(kc0, kc1):
                    nc.tensor.matmul(
                        psv[:],
                        xT[:, kc * 512 + pr * 128:kc * 512 + (pr + 1) * 128],
                        wv[:, kc * 1024 + g * 512:kc * 1024 + (g + 1) * 512],
                        start=(kc == 0), stop=(kc == KC - 1))
                if kc1 == KC:
                    dst = v65[pr][:].rearrange(
                        "p (m c) -> p m c", m=NM)[:, g * 8:(g + 1) * 8, 0:64]
                    nc.vector.tensor_copy(
                        dst, psv[:].rearrange("p (m c) -> p m c", m=8))

            def op_chip(pr, jb, c0, c1):
                key = ("o", pr, jb)
                if c0 == 0:
                    chip_ps[key] = pspool.tile([128, 512], F32, name="po",
                                               tag="pj", bufs=2)
                po = chip_ps[key]
                for c in range(c0, c1):
                    nc.tensor.matmul(
                        po[:],
                        ctx[pr][:, c * 128:(c + 1) * 128],
                        wo[:, c * 1024 + jb * 512:c * 1024 + (jb + 1) * 512],
                        start=(c == 0), stop=(c == KC - 1))
                if c1 == KC:
                    ot = otpool.tile([128, 512], F16, name="ot", tag="ot",
                                     bufs=4)
                    nc.vector.tensor_copy(ot[:], po[:])
                    nc.sync.dma_start(
                        out[pr * 128:(pr + 1) * 128,
                            jb * 512:(jb + 1) * 512], ot[:])

            def proj_qk(w_tile, c, dst_tiles, tag, scalar_ok):
                ps = pspool.tile([128, 512], F32, name=f"pp{c}", tag=tag,
                                 bufs=2 if tag in ("sc", "pj") else 1)
                for kc in range(KC):
                    nc.tensor.matmul(
                        ps[:],
                        w_tile[:, kc * 1024 + c * 128:kc * 1024 + (c + 1) * 128],
                        xT[:, kc * 512:(kc + 1) * 512],
                        start=(kc == 0), stop=(kc == KC - 1))
                for pr in range(NPAIR):
                    half_p = (pr % 2) * 64
                    dst_t = dst_tiles[pr // 2] if len(dst_tiles) == 2 \
                        else dst_tiles[pr]
                    for mp in range(2):
                        m = 2 * c + mp
                        src = ps[mp * 64:mp * 64 + 64,
                                 pr * 128:(pr + 1) * 128]
                        dst = dst_t[half_p:half_p + 64,
                                    m * 128:(m + 1) * 128]
                        if mp == 0 or not scalar_ok:
                            nc.vector.tensor_copy(dst, src)
                        else:
                            nc.scalar.copy(dst, src)

            def proj_v(pr, g, tag):
                psv = pspool.tile([128, 512], F32, name=f"pv{pr}{g}", tag=tag,
                                  bufs=2 if tag in ("sc", "pj") else 1)
                for kc in range(KC):
                    nc.tensor.matmul(
                        psv[:],
                        xT[:, kc * 512 + pr * 128:kc * 512 + (pr + 1) * 128],
                        wv[:, kc * 1024 + g * 512:kc * 1024 + (g + 1) * 512],
                        start=(kc == 0), stop=(kc == KC - 1))
                dst = v65[pr][:].rearrange("p (m c) -> p m c",
                                           m=NM)[:, g * 8:(g + 1) * 8, 0:64]
                src = psv[:].rearrange("p (m c) -> p m c", m=8)
                nc.vector.tensor_copy(dst, src)

            # ---------------- deferred pair-tail work ----------------
            cu = {}
            dens = {}
            state = {}

            def norm_recip(pr):
                """Batched reciprocal of pair pr's denominators + fp16
                cast + qs=3 relay (PE operand base must be 0/32/64)."""
                rec = cupool.tile([97, 512], F32, name="rec", tag="rec",
                                  bufs=2)
                nc.vector.reciprocal(rec[:], dens[pr][:])
                rec16 = cupool.tile([97, 512], F16, name="rec16", tag="r16",
                                    bufs=2)
                nc.vector.tensor_copy(rec16[:], rec[:])
                rec16b = cupool.tile([1, 512], F16, name="rec16b",
                                     tag="r16b", bufs=2)
                nc.vector.tensor_copy(rec16b[:], rec16[96:97, :])
                state[pr] = (rec16, rec16b)

            def norm_qs(pr, qs):
                """PE outer-product broadcast of 1/den + DVE multiply into
                the ctx layout."""
                rec16, rec16b = state[pr]
                pbs = pspool.tile([64, 512], F32, name="pbs", tag="pj",
                                  bufs=2)
                lo = qs * 32 if qs < 3 else 0
                rsrc = rec16[lo:lo + 1, :] if qs < 3 else rec16b[:]
                nc.tensor.matmul(pbs[:], ones64[lo:lo + 1, :], rsrc,
                                 start=True, stop=True)
                for p2 in range(2):
                    src = cu[(pr, qs)][:].rearrange(
                        "p (a q c) -> p a q c", a=2, q=2)[:, :, p2, :]
                    bb = pbs[:].rearrange(
                        "p (a q c) -> p a q c", a=2, q=2)[:, :, p2, :]
                    dst = ctx[pr][p2 * 64:(p2 + 1) * 64,
                                  qs * 256:(qs + 1) * 256].rearrange(
                        "p (a c) -> p a c", a=2)
                    nc.vector.tensor_mul(dst, src, bb)

            def outproj(pr, jb):
                po = pspool.tile([128, 512], F32, name="po", tag="pj",
                                 bufs=2)
                for c in range(KC):
                    nc.tensor.matmul(
                        po[:],
                        ctx[pr][:, c * 128:(c + 1) * 128],
                        wo[:, c * 1024 + jb * 512:c * 1024 + (jb + 1) * 512],
                        start=(c == 0), stop=(c == KC - 1))
                ot = otpool.tile([128, 512], F16, name="ot", tag="ot",
                                 bufs=4)
                nc.vector.tensor_copy(ot[:], po[:])
                nc.sync.dma_start(
                    out[pr * 128:(pr + 1) * 128,
                        jb * 512:(jb + 1) * 512], ot[:])

            # ---------------- phase 1 lead-in ----------------
            # k c0..3 in kc-halves (streams behind the split wkA DMA,
            # 4 open psums), then q c0..3, then v(pair0) lo half.
            KTAGS = ("sc", "sc", "pj", "pj")
            for c in range(4):
                qk_chip(wk, c, kTz, KTAGS[c], 0, 4, scalar_ok=True)
            for c in range(4):
                qk_chip(wk, c, kTz, KTAGS[c], 4, 8, scalar_ok=True)
            for c, tg in zip(range(4), ("pcA", "pcB", "sc", "sc")):
                qk_chip(wq, c, qT2, tg, 0, 8, scalar_ok=True)
            v_chip(0, 0, "pj", 0, 8)

            # ------------- static fill schedule (pass, mk) -> thunks -----
            # Every chip is <= 8 matmuls; placed so its inputs (DMA pieces,
            # the DVE reciprocal chain, normalized ctx) are ready before
            # the PE FIFO reaches it.
            SCHED = {}

            def at(pi, mk, fn):
                SCHED.setdefault((pi, mk), []).append(fn)

            # pass 0 (pr0-h0): k hi-chunks (wkB-gated), v0 hi, q hi
            for i, c in enumerate(range(4, 8)):
                at(0, 4 + i, lambda c=c: qk_chip(wk, c, kTz, "pj", 0, 8))
            at(0, 5, lambda: v_chip(0, 1, "pj", 0, 4))
            at(0, 6, lambda: v_chip(0, 1, "pj", 4, 8))
            for i, c in enumerate(range(4, 8)):
                at(0, 8 + 2 * i, lambda c=c: qk_chip(wq, c, qT2, "pj", 0, 8))
            # v projections for later pairs, two chips each
            for p in range(1, NPAIR):
                pi0 = (p - 1) * 2 + 1          # previous pair's h1 pass
                at(pi0, 1, lambda p=p: v_chip(p, 0, "pj", 0, 4))
                at(pi0, 2, lambda p=p: v_chip(p, 0, "pj", 4, 8))
                at(pi0, 8, lambda p=p: v_chip(p, 1, "pj", 0, 4))
                at(pi0, 9, lambda p=p: v_chip(p, 1, "pj", 4, 8))
            # previous pair's normalize + out-projection (recip chain is
            # kicked off at this pass's mk0; ready by ~mk5)
            for pr in range(NPAIR - 1):
                pi = (pr + 1) * 2
                for qs in range(4):
                    at(pi, 6 + qs, lambda pr=pr, qs=qs: norm_qs(pr, qs))
                at(pi, 10, lambda pr=pr: op_chip(pr, 0, 0, 4))
                at(pi, 11, lambda pr=pr: op_chip(pr, 0, 4, 8))
                at(pi, 12, lambda pr=pr: op_chip(pr, 1, 0, 4))
                at(pi, 13, lambda pr=pr: op_chip(pr, 1, 4, 8))

            # ------------- attention: 8 skewed passes --------------------
            def drain_pass(pc, pr, h2):
                """Drains of a finished pass: den rows (scalar) first so
                the DVE reciprocal chain can start, then ctx rows (DVE)."""
                if h2 == 0:
                    dens[pr] = cupool.tile([97, 512], F32, name="den",
                                           tag="den", bufs=2)
                for qh in range(2):
                    qs = h2 * 2 + qh
                    nc.scalar.copy(dens[pr][qs * 32:qs * 32 + 1, :],
                                   pc[qh][64:65, :])
                if h2 == 1:
                    norm_recip(pr)
                for qh in range(2):
                    qs = h2 * 2 + qh
                    c_t = cupool.tile([64, 512], F32, name=f"cu{pr}{qs}",
                                      tag=f"cu{qs}", bufs=2)
                    nc.vector.tensor_copy(c_t[:], pc[qh][0:64, :])
                    cu[(pr, qs)] = c_t

            pend_ctx = None
            prev_pass = None
            for pi in range(2 * NPAIR):
                pr, h2 = pi // 2, pi % 2
                t = pr // 2
                pc = [pspool.tile([65, 512], F32, name=f"pc{qh}",
                                  tag=("pcA", "pcB")[qh], bufs=1)
                      for qh in range(2)]
                for mk in range(NM):
                    sw = pspool.tile([128, 1024], F32, name="sw",
                                     tag="sc", bufs=2)
                    for qh in range(2):
                        nc.tensor.matmul(
                            sw[:, qh * 512:(qh + 1) * 512],
                            kTz[pr][:, mk * 128:(mk + 1) * 128],
                            qT2[t][:, (h2 * 1024 + qh * 512):
                                   (h2 * 1024 + (qh + 1) * 512)],
                            start=True, stop=True)
                    pT = ptpool.tile([128, 1024], F16, name="pT",
                                     tag="pt", bufs=4)
                    nc.scalar.activation(
                        pT[:], sw[:], mybir.ActivationFunctionType.Exp,
                        scale=float(SCALE))
                    # one-iteration skew: emit the PREVIOUS mk's ctx after
                    # this mk's scores, so the in-order PE FIFO never
                    # parks on the exp semaphore ahead of ready work.
                    if pend_ctx is not None:
                        pend_ctx()

                    def make_ctx(pc=pc, pr=pr, mk=mk, pT=pT):
                        for qh in range(2):
                            nc.tensor.matmul(
                                pc[qh][:],
                                v65[pr][:, mk * 65:(mk + 1) * 65],
                                pT[:, qh * 512:(qh + 1) * 512],
                                start=(mk == 0), stop=(mk == NM - 1))
                    pend_ctx = make_ctx
                    if mk == 0 and prev_pass is not None:
                        drain_pass(*prev_pass)
                    for fn in SCHED.get((pi, mk), []):
                        fn()
                prev_pass = (pc, pr, h2)

            # flush: last ctx, last drains, pair-3 tail
            pend_ctx()
            drain_pass(*prev_pass)
            for qs in range(4):
                norm_qs(3, qs)
            op_chip(3, 0, 0, 8)
            op_chip(3, 1, 0, 8)

    nc.compile()
    return nc


_CACHE = {}


def _get_nc():
    if "nc" not in _CACHE:
        _CACHE["nc"] = _build()
    return _CACHE["nc"]


def _kc_block(a, cols):
    """[1024, cols] -> [128, 8*cols] with kc blocks along columns."""
    return np.ascontiguousarray(
        a.reshape(KC, 128, cols).transpose(1, 0, 2).reshape(128, KC * cols))


def _prep_inputs(x, Wq, Wk, Wv, Wo):
    x = np.asarray(x, dtype=np.float32)
    wqa = _kc_block(np.ascontiguousarray(Wq.T, dtype=np.float16), 1024)
    wka = _kc_block(np.ascontiguousarray(Wk.T, dtype=np.float16), 1024)
    wva = _kc_block(np.ascontiguousarray(Wv.T, dtype=np.float16), 1024)
    woa = _kc_block(np.ascontiguousarray(Wo.T, dtype=np.float16), 1024)

    in_maps = []
    for core in range(NCORES):
        b, hg = core // 4, core % 4
        rows = x[b, hg * 512:(hg + 1) * 512, :]
        xTa = _kc_block(np.ascontiguousarray(rows.T.astype(np.float16)), 512)
        in_maps.append({
            "xTa": xTa, "wqa": wqa, "wka": wka, "wva": wva, "woa": woa,
        })
    return in_maps


def _run(in_maps, trace=False):
    nc = _get_nc()
    return run_bass_kernel_spmd(nc, in_maps, core_ids=list(range(NCORES)),
                                trace=trace)


def kernel(x, Wq, bq, Wk, bk, Wv, bv, Wo, bo, _trace=False):
    x = np.asarray(x, dtype=np.float32)
    in_maps = _prep_inputs(x, np.asarray(Wq), np.asarray(Wk),
                           np.asarray(Wv), np.asarray(Wo))
    res = _run(in_maps, trace=_trace)
    out = np.empty((B, S, D), dtype=np.float32)
    for core in range(NCORES):
        b, hg = core // 4, core % 4
        out[b, hg * 512:(hg + 1) * 512, :] = res.results[core]["out"]
    out += np.asarray(bo, dtype=np.float32)[None, None, :]
    kernel.last_result = res
    return out
